# revision 71
# baseline (speedup 1.0000x reference)
"""Grouped-Query Attention (B=2, S=2048, E=2048, 32 q heads, 8 kv heads, d=64)
on 8 Trainium2 NeuronCores.

Sharding: 8 cores = 2 batches x 4 kv-head-groups. Each core handles one batch
and 2 kv heads (= 8 q heads), computing its slice of attention plus the
row-parallel partial out-projection. The host sums the 4 partial outputs per
batch (no on-device collectives needed) and adds the output bias.

On-device pipeline per core (matmuls bf16, fp32 accumulation):
  xT (host-pretransposed, S-chunked) -> kT/qT/v projections ->
  scoresT = k @ qT (t on partitions) -> exp on ScalarE (scale 1/8 folded in,
  no max-subtraction: scores are O(5) for unit-variance inputs) ->
  AV^T: pav[q, d|denom] += exps_tile.T @ [v | ones]  (the ones column gives
  the softmax denominator directly, per (q, head), no cross-partition
  reduction needed) -> reciprocal + normalize (DVE) -> PE transpose back to
  [d, q] -> out-projection, software-pipelined one block behind attention.
"""

import sys

sys.path.insert(0, "/opt/trn_rl_repo")

import numpy as np
import ml_dtypes

BF16 = ml_dtypes.bfloat16

P = 128
B, S, E = 2, 2048, 2048
NUM_HEADS, NUM_KV_HEADS, HEAD_DIM = 32, 8, 64
GROUP = NUM_HEADS // NUM_KV_HEADS  # 4
NE = E // P  # 16 e-tiles (contraction tiles for projections)
NT = S // P  # 16 t-tiles (key/value positions)
NJ = GROUP  # 4 q-heads per kv head
SB = 128  # query-block size
NSB = S // SB  # 16 query blocks
NCH = 4  # xT S-chunks
SCALE = 1.0 / np.sqrt(HEAD_DIM)

_compiled = None  # cached program
_RUN_KWARGS = {}  # test harness may set e.g. {"trace": True}
_last_run = None  # BassKernelResults of the most recent kernel() call
PHASE = [""]  # build-time phase label, for the analysis tooling


def build_gqa_program():
    from concourse import bacc, mybir, tile, masks

    f32 = mybir.dt.float32
    bf16 = mybir.dt.bfloat16
    Exp = mybir.ActivationFunctionType.Exp

    nc = bacc.Bacc(None, target_bir_lowering=False, debug=False)
    with tile.TileContext(nc) as tc:
        with tc.tile_pool(name="dram", bufs=1, space="DRAM") as dram:
            xT = dram.tile([P, NCH, NE, 512], bf16, kind="ExternalInput", name="xT", uniquify=False)
            wq = dram.tile([P, NJ, NE, 128], bf16, kind="ExternalInput", name="wq", uniquify=False)
            wk = dram.tile([P, NE, 128], bf16, kind="ExternalInput", name="wk", uniquify=False)
            wv = dram.tile([P, NE, 128], bf16, kind="ExternalInput", name="wv", uniquify=False)
            wo = dram.tile([P, NJ, E], bf16, kind="ExternalInput", name="wo", uniquify=False)
            bqd = dram.tile([P, NJ], f32, kind="ExternalInput", name="bqd", uniquify=False)
            bkd = dram.tile([P, 1], f32, kind="ExternalInput", name="bkd", uniquify=False)
            bvd = dram.tile([P, 128], f32, kind="ExternalInput", name="bvd", uniquify=False)
            y = dram.tile([P, NT, E], bf16, kind="ExternalOutput", name="y", uniquify=False)

            with (
                tc.tile_pool(name="win", bufs=1) as win,
                tc.tile_pool(name="proj", bufs=1) as proj,
                tc.tile_pool(name="attn", bufs=2) as attn,
                tc.tile_pool(name="misc", bufs=2) as misc,
                tc.tile_pool(name="ps", bufs=2, space="PSUM") as ps,
            ):
                # ---- input DMAs (wk+xT on sync/gpsimd; wq+biases on ACT's
                # queue, which is idle until the first exp anyway) ----
                wk_sb = win.tile([P, NE, 128], bf16)
                nc.sync.dma_start(out=wk_sb[:, 0:2], in_=wk[:, 0:2])
                nc.gpsimd.dma_start(out=wk_sb[:, 2:], in_=wk[:, 2:])
                wq_sb = win.tile([P, NJ, NE, 128], bf16)
                bq_sb = win.tile([P, NJ], f32)
                nc.scalar.dma_start(out=bq_sb[:], in_=bqd[:])
                bk_sb = win.tile([P, 1], f32)
                nc.scalar.dma_start(out=bk_sb[:], in_=bkd[:])
                bv_sb = win.tile([P, 128], f32)
                nc.scalar.dma_start(out=bv_sb[:], in_=bvd[:])
                xT_sb = win.tile([P, NCH, NE, 512], bf16)
                nc.sync.dma_start(out=xT_sb[:, 0, 0:1], in_=xT[:, 0, 0:1])
                nc.sync.dma_start(out=xT_sb[:, 0, 1:4], in_=xT[:, 0, 1:4])
                nc.gpsimd.dma_start(out=xT_sb[:, 0, 4:10], in_=xT[:, 0, 4:10])
                nc.scalar.dma_start(out=xT_sb[:, 0, 13:16], in_=xT[:, 0, 13:16])
                nc.sync.dma_start(out=xT_sb[:, 0, 10:13], in_=xT[:, 0, 10:13])
                for j4 in range(4):
                    nc.scalar.dma_start(out=wq_sb[:, j4], in_=wq[:, j4])
                wv_done = False
                for c in range(1, NCH):
                    nc.sync.dma_start(out=xT_sb[:, c, 0:8], in_=xT[:, c, 0:8])
                    nc.gpsimd.dma_start(out=xT_sb[:, c, 8:16], in_=xT[:, c, 8:16])
                    if c == 1 and not wv_done:
                        wv_sb = win.tile([P, NE, 128], bf16)
                        nc.sync.dma_start(out=wv_sb[:], in_=wv[:])
                        wv_done = True
                wo_sb = win.tile([P, NJ, E], bf16)
                nc.gpsimd.dma_start(out=wo_sb[:], in_=wo[:])

                # identity for PE transposes + exp table warm-up
                ident = win.tile([P, P], bf16)
                masks.make_identity(nc, ident[:])
                warm = misc.tile([1, 1], f32, tag="warm")
                nc.scalar.activation(out=warm[:], in_=ident[0:1, 0:1], func=Exp)

                # v with a ones column per kv head: [v_g | 1] -> denominator
                # comes out of the AV matmul for free.
                v_aug = [win.tile([P, NT, 65], bf16, name=f"vaug{g}") for g in range(2)]
                for g in range(2):
                    nc.gpsimd.memset(v_aug[g][:, :, 64:65], 1.0)

                # ---- projections (kT first; qT n=0; rest interleaved) ----
                kT = proj.tile([P, S], bf16)

                def k_proj(n):
                    pk = ps.tile([P, 512], f32, tag="sc", name="pk")
                    for et in range(NE):
                        nc.tensor.matmul(
                            pk[:], wk_sb[:, et], xT_sb[:, n, et],
                            start=(et == 0), stop=(et == NE - 1),
                        )
                    nc.vector.tensor_scalar_add(
                        out=kT[:, n * 512:(n + 1) * 512], in0=pk[:],
                        scalar1=bk_sb[:, 0:1],
                    )

                qT = proj.tile([P, NJ, S], bf16)

                def q_proj(n, j):
                    for fn in q_proj_quanta(n, j):
                        fn()

                def q_proj_quanta(n, j):
                    """q-proj unit as 4 self-contained quanta (128 cols each)."""

                    def quarter(qi):
                        def run():
                            pq = ps.tile([P, 128], f32, tag="trpy", bufs=2, name="pq")
                            cl = qi * 128
                            for et in range(NE):
                                nc.tensor.matmul(
                                    pq[:], wq_sb[:, j, et],
                                    xT_sb[:, n, et, cl:cl + 128],
                                    start=(et == 0), stop=(et == NE - 1),
                                )
                            nc.vector.tensor_scalar_add(
                                out=qT[:, j, n * 512 + cl:n * 512 + cl + 128],
                                in0=pq[:], scalar1=bq_sb[:, j:j + 1],
                            )
                        return run

                    return [quarter(qi) for qi in range(4)]

                def v_proj(tt):
                    pv = ps.tile([P, 128], f32, tag="trpy", bufs=2, name="pv")
                    c, sl = tt // 4, (tt % 4) * 128
                    for et in range(NE):
                        nc.tensor.matmul(
                            pv[:], xT_sb[:, c, et, sl:sl + 128], wv_sb[:, et],
                            start=(et == 0), stop=(et == NE - 1),
                        )
                    for g in range(2):
                        nc.vector.tensor_add(
                            out=v_aug[g][:, tt, 0:64],
                            in0=pv[:, g * 64:(g + 1) * 64],
                            in1=bv_sb[:, g * 64:(g + 1) * 64],
                        )

                def alloc_exps():
                    return [
                        attn.tile([P, NT, NJ, SB], bf16, tag=f"exp{g}", name=f"exp{g}")
                        for g in range(2)
                    ]

                def scores_half(sb, exps, half, popper=None):
                    """One t-half of scores + exp for query block sb."""
                    ssl = slice(sb * SB, (sb + 1) * SB)
                    for grp in range(2):
                        poff = grp * 64
                        sc = ps.tile([P, 2, NJ, SB], f32, tag="sc", name="sc")
                        for q in range(2):
                            tt = 2 * half + q
                            nc.tensor.matmul(
                                sc[:, q],
                                kT[poff:poff + 64, tt * 128:(tt + 1) * 128],
                                qT[poff:poff + 64, :, ssl],
                                start=True, stop=True,
                            )
                        nc.scalar.activation(
                            out=exps[grp][:, 2 * half:2 * half + 2],
                            in_=sc[:], func=Exp, scale=float(SCALE),
                        )
                        if popper is not None:
                            popper()

                def av_chunk(pav, exps, grp, j):
                    """One (grp, head) accumulation group: full t contraction.
                    Groups must run one-at-a-time per PSUM bank."""
                    for tt in range(NT):
                        nc.tensor.matmul(
                            pav[grp][:, j],
                            exps[grp][:, tt, j],
                            v_aug[grp][:, tt],
                            start=(tt == 0), stop=(tt == NT - 1),
                            skip_group_check=True,
                        )

                def normalize_muls(pav):
                    """DVE-only: 1/denom + scale; frees the pav PSUM tiles."""
                    recs = []
                    for grp in range(2):
                        rec = misc.tile([P, NJ], f32, tag=f"rec{grp}", bufs=2, name="rec")
                        nc.vector.reciprocal(out=rec[:], in_=pav[grp][:, :, 64:65])
                        recs.append(rec)
                    aosbs = []
                    for j in range(NJ):
                        aosb = misc.tile([P, 2, 64], bf16, tag="aosb", bufs=8, name="aosb")
                        for grp in range(2):
                            nc.vector.tensor_scalar_mul(
                                out=aosb[:, grp], in0=pav[grp][:, j, 0:64],
                                scalar1=recs[grp][:, j:j + 1],
                            )
                        aosbs.append(aosb)
                    return aosbs

                def transpose_quantum(aosbs, aoTt):
                    for j in range(NJ):
                        tr = ps.tile([P, SB], bf16, tag="trpy", bufs=2, name="tr")
                        nc.tensor.transpose(tr[:], aosbs[j][:], ident[:])
                        nc.vector.tensor_copy(out=aoTt[:, j], in_=tr[:])

                def out_proj_unit(sb, aoTt, n):
                    py = ps.tile([P, 512], f32, tag="trpy", bufs=2, name="py")
                    for j in range(NJ):
                        nc.tensor.matmul(
                            py[:], aoTt[:, j],
                            wo_sb[:, j, n * 512:(n + 1) * 512],
                            start=(j == 0), stop=(j == NJ - 1),
                        )
                    ysb = misc.tile([P, 512], bf16, tag="ysb", bufs=3, name="ysb")
                    nc.vector.tensor_copy(out=ysb[:], in_=py[:])
                    eng = nc.sync if n % 2 == 0 else nc.gpsimd
                    eng.dma_start(out=y[:, sb, n * 512:(n + 1) * 512], in_=ysb[:])

                # ---- rolling work queue ----
                # Entries: dict(cost, fn, av_iter=None, deadline=None).
                # av quanta only eligible once their iter's scores are past
                # h=0 (exp tiles ready); deadline=b quanta must pop before
                # scores(b) is emitted (they produce its qT columns).
                workq = []
                state = {"iter": -1, "h": 0}

                def eligible(ent):
                    if ent["hold_iter"] is not None and state["iter"] < ent["hold_iter"]:
                        return False
                    if ent["av_iter"] is not None:
                        # exp tiles ready (h>=1) and no AV prerequisites left
                        if state["iter"] <= ent["av_iter"] and state["h"] < 1:
                            return False
                        return not any(e["before_av"] for e in workq)
                    return True

                def pop_quanta(budget):
                    spent = 0
                    while workq and spent < budget:
                        idx = next(
                            (k for k, e in enumerate(workq) if eligible(e)), None
                        )
                        if idx is None:
                            break
                        e = workq.pop(idx)
                        e["fn"]()
                        spent += e["cost"]

                def flush(pred):
                    for e in [e for e in workq if pred(e)]:
                        workq.remove(e)
                        e["fn"]()

                def push(cost, fn, av_iter=None, deadline=None, before_av=False,
                         hold_iter=None):
                    workq.append(
                        dict(cost=cost, fn=fn, av_iter=av_iter, deadline=deadline,
                             before_av=before_av, hold_iter=hold_iter)
                    )

                def q_block(b):
                    """quanta producing qT columns for scores block b."""
                    n, qi = b // 4, b % 4
                    return [q_proj_quanta(n, j)[qi] for j in range(NJ)]

                # ---- prologue: k/q for block 0, then scores(0) over fillers,
                # with the remaining kT tiles produced just-in-time ----
                k_proj(0)
                for fn in q_block(0):
                    fn()
                for fn in q_block(1):
                    push(860, fn, deadline=1)
                for tt in range(NT):
                    push(880, (lambda tt=tt: v_proj(tt)), before_av=True)
                exps_cur = alloc_exps()
                for h in range(8):
                    if h in (2, 4, 6):
                        k_proj(h // 2)
                    scores_half(0, exps_cur, h, lambda: pop_quanta(700))

                # ---- software-pipelined main loop ----
                ao_prev = None
                for i in range(NSB):
                    PHASE[0] = f"it{i:02d}"
                    state["iter"], state["h"] = i, 0
                    has_next = i + 1 < NSB
                    exps_next = alloc_exps() if has_next else None
                    pav = [
                        ps.tile([P, NJ, 65], f32, tag=f"pav{g}", bufs=1, name=f"pav{g}")
                        for g in range(2)
                    ]
                    ecur = exps_cur
                    for g in range(2):
                        for j in range(NJ):
                            push(
                                450,
                                (lambda g=g, j=j, pav=pav, e=ecur: av_chunk(pav, e, g, j)),
                                av_iter=i,
                            )
                    if ao_prev is not None:
                        ao = ao_prev
                        for n in range(4):
                            push(860, (lambda n=n, ao=ao, s=i - 1: out_proj_unit(s, ao, n)),
                                 hold_iter=None)
                    b = i + 2
                    if b <= NSB - 1:
                        for fn in q_block(b):
                            push(860, fn, deadline=b,
                                 hold_iter=(14 if b == 15 else None))

                    if has_next:
                        flush(lambda e: e["deadline"] is not None and e["deadline"] <= i + 1)
                        for h in range(8):
                            state["h"] = h
                            scores_half(i + 1, exps_next, h, lambda: pop_quanta(700))
                    # block i's AV must complete before its normalize (DVE,
                    # frees pav); the PE transposes are deferred into the next
                    # iteration's queue so ACT keeps eating across the boundary.
                    state["h"] = 8
                    if not has_next:
                        # last iteration: drain ready non-AV work first so it
                        # isn't ordered behind the ACT-gated av(15) chain in
                        # the trpy PSUM rotation
                        flush(lambda e: e["av_iter"] is None)
                    flush(lambda e: e["av_iter"] == i)
                    aosbs = normalize_muls(pav)
                    aoTt = attn.tile([P, NJ, SB], bf16, tag="aoT", bufs=2, name="aoTt")
                    if has_next:
                        push(300, (lambda a=aosbs, t=aoTt: transpose_quantum(a, t)))
                        ao_prev = aoTt
                    else:
                        last_aosbs, last_ao = aosbs, aoTt
                    exps_cur = exps_next
                flush(lambda e: True)
                # ---- epilogue for block 15: interleave the transposes with
                # the first two out-proj accumulations (the sc PSUM tag is
                # idle now), then finish n=2,3 with the last y DMA split
                # across both queues to shorten the drain. ----
                pys = [
                    ps.tile([P, 512], f32, tag="sc", name=f"pyl{n}") for n in range(2)
                ]
                for j in range(NJ):
                    tr = ps.tile([P, SB], bf16, tag="trpy", bufs=2, name="tr")
                    nc.tensor.transpose(tr[:], last_aosbs[j][:], ident[:])
                    nc.vector.tensor_copy(out=last_ao[:, j], in_=tr[:])
                    for n in range(2):
                        nc.tensor.matmul(
                            pys[n][:], last_ao[:, j],
                            wo_sb[:, j, n * 512:(n + 1) * 512],
                            start=(j == 0), stop=(j == NJ - 1),
                        )
                for n in range(2):
                    ysb = misc.tile([P, 512], bf16, tag="ysb", bufs=3, name="ysb")
                    nc.vector.tensor_copy(out=ysb[:], in_=pys[n][:])
                    eng = nc.sync if n % 2 == 0 else nc.gpsimd
                    eng.dma_start(out=y[:, NSB - 1, n * 512:(n + 1) * 512], in_=ysb[:])
                out_proj_unit(NSB - 1, last_ao, 2)
                py3 = ps.tile([P, 512], f32, tag="trpy", bufs=2, name="py3")
                for j in range(NJ):
                    nc.tensor.matmul(
                        py3[:], last_ao[:, j], wo_sb[:, j, 3 * 512:4 * 512],
                        start=(j == 0), stop=(j == NJ - 1),
                    )
                ysb3 = misc.tile([P, 512], bf16, tag="ysb", bufs=3, name="ysb3")
                nc.vector.tensor_copy(out=ysb3[:], in_=py3[:])
                nc.sync.dma_start(out=y[:, NSB - 1, 3 * 512:3 * 512 + 256], in_=ysb3[:, 0:256])
                nc.gpsimd.dma_start(out=y[:, NSB - 1, 3 * 512 + 256:4 * 512], in_=ysb3[:, 256:512])
    nc.compile()
    return nc


def _get_program():
    global _compiled
    if _compiled is None:
        _compiled = build_gqa_program()
    return _compiled


def _wrap_pmn(a2d, ntile):
    """[R, C] -> [128, R/128, C] with row r at (r % 128, r // 128)."""
    r, c = a2d.shape
    return np.ascontiguousarray(a2d.reshape(ntile, P, c).transpose(1, 0, 2))


def shard_inputs(x, Wq, bq, Wk, bk, Wv, bv, Wo):
    """Build the 8 per-core input maps (host-side shard + transpose + cast)."""
    ins = []
    for c in range(8):
        b, g = c // 4, c % 4
        # q-head columns for this core, ordered (j, pair, d):
        # global q-col = (2g + pair) * 256 + j * 64 + d
        j_idx, pair_idx, d_idx = np.meshgrid(
            np.arange(NJ), np.arange(2), np.arange(64), indexing="ij"
        )
        qcols = ((2 * g + pair_idx) * (GROUP * 64) + j_idx * 64 + d_idx).reshape(-1)
        kvcols = np.arange(g * 128, (g + 1) * 128)  # kv heads 2g, 2g+1

        xT = np.ascontiguousarray(x[b].T)  # [E, S] f32
        xTw = _wrap_pmn(xT, NE)  # [128, NE, S]
        xTc = np.ascontiguousarray(
            xTw.reshape(P, NE, NCH, 512).transpose(0, 2, 1, 3)
        )  # [128, NCH, NE, 512]
        ins.append(
            {
                "xT": xTc.astype(BF16),
                "wq": np.ascontiguousarray(
                    _wrap_pmn(Wq[:, qcols], NE)
                    .reshape(P, NE, NJ, 128)
                    .transpose(0, 2, 1, 3)
                ).astype(BF16),
                "wk": _wrap_pmn(Wk[:, kvcols], NE).astype(BF16),
                "wv": _wrap_pmn(Wv[:, kvcols], NE).astype(BF16),
                "wo": _wrap_pmn(Wo[qcols, :], NJ).astype(BF16),
                "bqd": np.ascontiguousarray(
                    bq[qcols].reshape(NJ, P).T.astype(np.float32)
                ),
                "bkd": bk[kvcols].reshape(P, 1).astype(np.float32),
                "bvd": np.ascontiguousarray(
                    np.broadcast_to(bv[kvcols][None, :], (P, 128))
                ).astype(np.float32),
            }
        )
    return ins


def gather_outputs(results, bo):
    """Sum the 4 row-parallel partials per batch, add bias."""
    y = np.zeros((B, S, E), np.float64)
    for c in range(8):
        b = c // 4
        part = results[c]["y"]  # [128, NT, E] bf16
        y[b] += part.transpose(1, 0, 2).reshape(S, E).astype(np.float64)
    return (y + bo.astype(np.float64)).astype(np.float32)


def kernel(x, Wq, bq, Wk, bk, Wv, bv, Wo, bo):
    from concourse.bass_utils import run_bass_kernel_spmd

    x = np.asarray(x, np.float32)
    nc = _get_program()
    ins = shard_inputs(
        x,
        np.asarray(Wq, np.float32),
        np.asarray(bq, np.float32),
        np.asarray(Wk, np.float32),
        np.asarray(bk, np.float32),
        np.asarray(Wv, np.float32),
        np.asarray(bv, np.float32),
        np.asarray(Wo, np.float32),
    )
    r = run_bass_kernel_spmd(nc, ins, list(range(8)), **_RUN_KWARGS)
    globals()["_last_run"] = r
    return gather_outputs(r.results, np.asarray(bo, np.float32))
